# revision 1
# baseline (speedup 1.0000x reference)
"""GQA + sliding-window attention (B=2, S=2048, E=2048, HQ=16, HKV=4, D=128, W=512).

Sharding: 8 cores = 2 batches x 4 KV-head groups (tensor parallel).
Each core computes its batch's full sequence for one KV head + its 4 Q heads,
plus the (row-sharded) output projection partial; the host sums the 4 partials
per batch (the "all-reduce" done host-side) and adds bo.

On-device layout (per core):
  xT   [E, S]   bf16  (x[b] transposed on host)
  wqT  [E, 512] bf16  (Wq rows of this group, transposed)
  wkT  [E, 128] bf16  (pre-scaled by 1/sqrt(D))
  wvT  [E, 128] bf16
  woT  [512, E] bf16  (Wo cols of this group, transposed)
  cosT/sinT [128, S] f32 RoPE tables (sinT sign-folded for rotate-half)
  out  [S, E]   f32   partial output

Pipeline: projections (Q^T,K^T with fused RoPE; V natural layout) ->
scores S^T[k,q] per K-tile row (banded: 5 q-tiles per k-tile) -> additive
masks on the two partial tiles -> exp (ScalarE, psum->sbuf bf16) ->
PV matmuls (V stationary) + row-sum matmuls (ones stationary, sharing the
PSUM bank via has_written semantics) -> normalize O^T columns via
reciprocal + partition-broadcast -> output projection -> DMA out.
"""

import os

import numpy as np
import ml_dtypes

import concourse.bass as bass
import concourse.mybir as mybir
import concourse.tile as tile
from concourse.tile import add_dep_helper
from concourse.bass_utils import run_bass_kernel_spmd

B, S, E = 2, 2048, 2048
HQ, HKV, D = 16, 4, 128
WINDOW = 512
ROPE_BASE = 10000.0
N_CORES = 8
GROUP = HQ // HKV          # 4 Q heads per KV head
HD_Q = GROUP * D           # 512
ST = S // 128              # 16 sequence tiles
KTILES = E // 128          # 16 contraction tiles over E
WT = WINDOW // 128         # 4 -> window spans WT+1 = 5 q-tiles
NEG = -30000.0

f32 = mybir.dt.float32
bf16 = mybir.dt.bfloat16


def _split_sync_waits(nc, max_waits=1):
    """walrus in this container rejects instructions with more than one
    sync-wait; split extras onto preceding same-engine NoOps."""
    for fn in nc.m.functions:
        for blk in fn.blocks:
            new_insts = []
            for inst in blk.instructions:
                si = getattr(inst, "sync_info", None)
                if si is not None and len(si.on_wait) > max_waits:
                    waits = list(si.on_wait)
                    head, tail = waits[:-max_waits], waits[-max_waits:]
                    for i in range(0, len(head), max_waits):
                        nop = mybir.InstNoOp(
                            name=f"splitwait-{nc.next_id()}",
                            ins=[], outs=[],
                            sync_info=mybir.SyncInfo(
                                on_wait=head[i:i + max_waits], on_update=[]),
                            bass_nofuse=True,
                        )
                        nop.engine = inst.engine
                        new_insts.append(nop)
                    inst.sync_info = mybir.SyncInfo(
                        on_wait=tail, on_update=list(si.on_update))
                new_insts.append(inst)
            blk.instructions[:] = new_insts


def build_kernel(has_bias):
    nc = bass.Bass("TRN2", target_bir_lowering=False, debug=False,
                   num_devices=N_CORES)
    Exp = mybir.ActivationFunctionType.Exp

    xT = nc.dram_tensor("xT", [E, S], bf16, kind="ExternalInput").ap()
    wqT = nc.dram_tensor("wqT", [E, HD_Q], bf16, kind="ExternalInput").ap()
    wkT = nc.dram_tensor("wkT", [E, D], bf16, kind="ExternalInput").ap()
    wvT = nc.dram_tensor("wvT", [E, D], bf16, kind="ExternalInput").ap()
    woT = nc.dram_tensor("woT", [HD_Q, E], bf16, kind="ExternalInput").ap()
    cosT = nc.dram_tensor("cosT", [D, S], f32, kind="ExternalInput").ap()
    sinT = nc.dram_tensor("sinT", [D, S], f32, kind="ExternalInput").ap()
    if has_bias:
        bqr = nc.dram_tensor("bqr", [1, HD_Q], bf16, kind="ExternalInput").ap()
        bkr = nc.dram_tensor("bkr", [1, D], bf16, kind="ExternalInput").ap()
        bvr = nc.dram_tensor("bvr", [1, D], bf16, kind="ExternalInput").ap()
    out = nc.dram_tensor("out", [S, E], f32, kind="ExternalOutput").ap()

    with tile.TileContext(nc) as tc:
        with tc.tile_pool(name="singles", bufs=1) as singles, \
             tc.tile_pool(name="upool", bufs=3) as upool, \
             tc.tile_pool(name="epool", bufs=7) as epool, \
             tc.tile_pool(name="rbpool", bufs=3) as rbpool, \
             tc.tile_pool(name="rhatpool", bufs=3) as rhatpool, \
             tc.tile_pool(name="ostage", bufs=3) as ostage:

            # ---- resident tensors ----
            xt = singles.tile([128, KTILES, S], bf16)
            wq = singles.tile([128, KTILES, HD_Q], bf16)
            wk = singles.tile([128, KTILES, D], bf16)
            wv = singles.tile([128, KTILES, D], bf16)
            wo = singles.tile([128, GROUP, E], bf16)
            cost = singles.tile([128, S], f32)
            sint = singles.tile([128, S], f32)
            qt = singles.tile([128, GROUP, S], bf16)
            kt = singles.tile([128, S], bf16)
            vv = singles.tile([128, ST, D], bf16)
            ot = singles.tile([128, GROUP * ST, D], bf16)
            m0 = singles.tile([128, 128], f32)
            m4 = singles.tile([128, 128], f32)
            onescol = singles.tile([128, 1], bf16)

            for t in range(KTILES):
                nc.sync.dma_start(out=xt[:, t, :], in_=xT[t * 128:(t + 1) * 128, :])
                nc.sync.dma_start(out=wq[:, t, :], in_=wqT[t * 128:(t + 1) * 128, :])
                nc.sync.dma_start(out=wk[:, t, :], in_=wkT[t * 128:(t + 1) * 128, :])
                nc.sync.dma_start(out=wv[:, t, :], in_=wvT[t * 128:(t + 1) * 128, :])
            for h in range(GROUP):
                nc.sync.dma_start(out=wo[:, h, :], in_=woT[h * 128:(h + 1) * 128, :])
            nc.sync.dma_start(out=cost[:], in_=cosT)
            nc.sync.dma_start(out=sint[:], in_=sinT)
            bq_t = bk_t = bv_t = onesrow = None
            if has_bias:
                bq_t = singles.tile([1, HD_Q], bf16)
                bk_t = singles.tile([1, D], bf16)
                bv_t = singles.tile([1, D], bf16)
                onesrow = singles.tile([1, 512], bf16)
                nc.sync.dma_start(out=bq_t[:], in_=bqr)
                nc.sync.dma_start(out=bk_t[:], in_=bkr)
                nc.sync.dma_start(out=bv_t[:], in_=bvr)
                nc.gpsimd.memset(onesrow[:], 1.0)

            # masks in S^T [k(p), q(x)] orientation:
            # diag tile: allowed iff q >= k  ->  x - p >= 0
            nc.gpsimd.memset(m0[:], 0.0)
            nc.gpsimd.affine_select(
                out=m0[:], in_=m0[:], compare_op=mybir.AluOpType.is_ge,
                fill=NEG, base=0, channel_multiplier=-1, pattern=[[1, 128]])
            # off-4 tile: allowed iff q <= k  ->  p - x >= 0
            nc.gpsimd.memset(m4[:], 0.0)
            nc.gpsimd.affine_select(
                out=m4[:], in_=m4[:], compare_op=mybir.AluOpType.is_ge,
                fill=NEG, base=0, channel_multiplier=1, pattern=[[-1, 128]])
            nc.gpsimd.memset(onescol[:], 1.0)

            with tc.tile_pool(name="qk_psum", bufs=2, space="PSUM") as qk_psum:

                def proj_rope(dst, wtile, m_off, btile):
                    """dst[:, :] = rope(W^T.T @ x^T); dst is a [128, S] view."""
                    for n in range(S // 512):
                        ps = qk_psum.tile([128, 512], f32, tag="ps")
                        for k in range(KTILES):
                            nc.tensor.matmul(
                                ps[:], wtile[:, k, m_off:m_off + 128],
                                xt[:, k, n * 512:(n + 1) * 512],
                                start=(k == 0),
                                stop=(k == KTILES - 1 and btile is None))
                        if btile is not None:
                            nc.tensor.matmul(
                                ps[:], btile[0:1, m_off:m_off + 128],
                                onesrow[0:1, :], start=False, stop=True)
                        u_t = upool.tile([128, 512], bf16, tag="u")
                        u_sh = upool.tile([128, 512], bf16, tag="ush")
                        sl = slice(n * 512, (n + 1) * 512)
                        nc.vector.tensor_mul(u_t[:], ps[:], sint[:, sl])
                        nc.sync.dma_start(out=u_sh[0:64, :], in_=u_t[64:128, :])
                        nc.sync.dma_start(out=u_sh[64:128, :], in_=u_t[0:64, :])
                        nc.vector.tensor_mul(dst[:, sl], ps[:], cost[:, sl])
                        nc.vector.tensor_add(dst[:, sl], dst[:, sl], u_sh[:])

                with tc.tile_pool(name="v_psum", bufs=2, space="PSUM") as v_psum:
                    # K^T projection + rope (pre-scaled by 1/sqrt(D) on host)
                    proj_rope(kt, wk, 0, bk_t)

                    # V in natural [s, d] layout (no rope)
                    for sm in range(ST):
                        psv = v_psum.tile([128, 128], f32, tag="vps")
                        for k in range(KTILES):
                            nc.tensor.matmul(
                                psv[:], xt[:, k, sm * 128:(sm + 1) * 128],
                                wv[:, k, :], start=(k == 0),
                                stop=(k == KTILES - 1 and not has_bias))
                        if has_bias:
                            nc.tensor.matmul(
                                psv[:], onesrow[0:1, 0:128],
                                bv_t[0:1, :], start=False, stop=True)
                        nc.scalar.copy(vv[:, sm, :], psv[:])

                    for m in range(GROUP):
                        proj_rope(qt[:, m, :], wq, m * 128, bq_t)

            with tc.tile_pool(name="score_psum", bufs=1, space="PSUM") as score_psum, \
                 tc.tile_pool(name="pv_psum", bufs=6, space="PSUM") as pv_psum:

                for m in range(GROUP):
                    e_tiles = {}
                    po_tiles = {}
                    pv0 = {}

                    def finish(qi):
                        po = po_tiles.pop(qi)
                        pv0.pop(qi, None)
                        rhat = rhatpool.tile([1, 128], f32, tag="rhat")
                        nc.vector.reciprocal(rhat[:], po[0:1, 128:256])
                        rb = rbpool.tile([128, 128], f32, tag="rb")
                        src = rhat[:]
                        bc = bass.AP(src.tensor, src.offset,
                                     [[1, 1], [0, 128], [1, 128]])
                        nc.sync.dma_start(out=rb[:], in_=bc)
                        nc.vector.tensor_mul(
                            ot[:, m * ST + qi, :], po[:, 0:128], rb[:])

                    def contrib(kj):
                        """PV + row-sum contributions of E_kj; V/ones each
                        loaded once per kj instead of 5x per q-tile."""
                        e_t = e_tiles[kj]
                        qis = list(range(kj, min(kj + WT, ST - 1) + 1))
                        for qi in qis:
                            first = (kj == max(0, qi - WT))
                            if first:
                                po_tiles[qi] = pv_psum.tile(
                                    [128, 256], f32, tag="po",
                                    name=f"po_{m}_{qi}")
                            off = (qi - kj) * 128
                            mm = nc.tensor.matmul(
                                po_tiles[qi][:, 0:128], vv[:, kj, :],
                                e_t[:, off:off + 128],
                                start=first, stop=(qi == kj))
                            if first:
                                pv0[qi] = mm
                        for qi in qis:
                            first = (kj == max(0, qi - WT))
                            off = (qi - kj) * 128
                            mm = nc.tensor.matmul(
                                po_tiles[qi][0:1, 128:256], onescol[:],
                                e_t[:, off:off + 128],
                                start=False, stop=(qi == kj),
                                skip_group_check=True)
                            if first:
                                # rT group relies on pv0's start=True having
                                # cleared the bank's has_written bits first
                                add_dep_helper(mm.ins, pv0[qi].ins, sync=False,
                                               reason="rT after bank clear")
                        finish(kj)

                    for kj in range(ST):
                        nw = min(WT + 1, ST - kj)
                        W = 128 * nw
                        q0 = kj * 128
                        pss = score_psum.tile([128, 640], f32, tag="ss")
                        n0 = min(W, 512)
                        nc.tensor.matmul(
                            pss[:, 0:n0], kt[:, kj * 128:(kj + 1) * 128],
                            qt[:, m, q0:q0 + n0], start=True, stop=True)
                        if W > 512:
                            nc.tensor.matmul(
                                pss[:, 512:W], kt[:, kj * 128:(kj + 1) * 128],
                                qt[:, m, q0 + 512:q0 + W], start=True, stop=True)
                        nc.vector.tensor_add(pss[:, 0:128], pss[:, 0:128], m0[:])
                        if nw == WT + 1:
                            nc.vector.tensor_add(
                                pss[:, 512:640], pss[:, 512:640], m4[:])
                        e_t = epool.tile([128, 640], bf16, tag="e")
                        nc.scalar.activation(e_t[:, 0:W], pss[:, 0:W], Exp)
                        e_tiles[kj] = e_t
                        if kj >= 1:
                            contrib(kj - 1)
                    contrib(ST - 1)

            # output projection: out[q, :] = sum_h O_h[q, :] @ WoT_h
            with tc.tile_pool(name="out_psum", bufs=2, space="PSUM") as out_psum:
                for qi in range(ST):
                    for ch in range(E // 512):
                        pso = out_psum.tile([128, 512], f32, tag="po2")
                        for h in range(GROUP):
                            nc.tensor.matmul(
                                pso[:], ot[:, h * ST + qi, :],
                                wo[:, h, ch * 512:(ch + 1) * 512],
                                start=(h == 0), stop=(h == GROUP - 1))
                        st = ostage.tile([128, 512], f32, tag="st")
                        if ch % 2 == 0:
                            nc.vector.tensor_copy(st[:], pso[:])
                        else:
                            nc.scalar.copy(st[:], pso[:])
                        nc.sync.dma_start(
                            out=out[qi * 128:(qi + 1) * 128, ch * 512:(ch + 1) * 512],
                            in_=st[:])

    _split_sync_waits(nc)
    return nc


def _rope_tables():
    half = D // 2
    inv_freq = 1.0 / (ROPE_BASE ** (np.arange(half, dtype=np.float64) / half))
    ang = np.arange(S, dtype=np.float64)[:, None] * inv_freq[None, :]  # [S, 64]
    cos = np.cos(ang).T.astype(np.float32)          # [64, S]
    sin = np.sin(ang).T.astype(np.float32)
    cosT = np.concatenate([cos, cos], 0)            # [128, S]
    sinT = np.concatenate([sin, -sin], 0)           # sign-folded rotate-half
    return np.ascontiguousarray(cosT), np.ascontiguousarray(sinT)


def kernel(x, Wq, bq, Wk, bk, Wv, bv, Wo, bo, **kw):
    x = np.asarray(x, np.float32)
    Wq = np.asarray(Wq, np.float32); bq = np.asarray(bq, np.float32)
    Wk = np.asarray(Wk, np.float32); bk = np.asarray(bk, np.float32)
    Wv = np.asarray(Wv, np.float32); bv = np.asarray(bv, np.float32)
    Wo = np.asarray(Wo, np.float32); bo = np.asarray(bo, np.float32)

    has_bias = bool(np.any(bq) or np.any(bk) or np.any(bv))
    nc = build_kernel(has_bias)

    bff = ml_dtypes.bfloat16
    cosT, sinT = _rope_tables()
    scale = 1.0 / np.sqrt(np.float32(D))

    in_maps = []
    for c in range(N_CORES):
        b, h = c // HKV, c % HKV
        qs = slice(h * HD_Q, (h + 1) * HD_Q)
        ks = slice(h * D, (h + 1) * D)
        m = {
            "xT": np.ascontiguousarray(x[b].T).astype(bff),
            "wqT": np.ascontiguousarray(Wq[qs].T).astype(bff),
            "wkT": np.ascontiguousarray((Wk[ks] * scale).T).astype(bff),
            "wvT": np.ascontiguousarray(Wv[ks].T).astype(bff),
            "woT": np.ascontiguousarray(Wo[:, qs].T).astype(bff),
            "cosT": cosT,
            "sinT": sinT,
        }
        if has_bias:
            m["bqr"] = np.ascontiguousarray(bq[qs][None, :]).astype(bff)
            m["bkr"] = np.ascontiguousarray((bk[ks] * scale)[None, :]).astype(bff)
            m["bvr"] = np.ascontiguousarray(bv[ks][None, :]).astype(bff)
        in_maps.append(m)

    res = run_bass_kernel_spmd(nc, in_maps, core_ids=list(range(N_CORES)))
    global LAST_RESULT
    LAST_RESULT = res
    if os.environ.get("BASS_KERNEL_RETIME"):
        # executable is now cached in-process: a second run times
        # transfer + device execution without compile.
        import time
        t0 = time.time()
        run_bass_kernel_spmd(nc, in_maps, core_ids=list(range(N_CORES)))
        print(f"retime run (transfer+exec): {time.time()-t0:.3f}s")

    out_full = np.zeros((B, S, E), np.float32)
    for c in range(N_CORES):
        out_full[c // HKV] += res.results[c]["out"]
    out_full += bo[None, None, :]
    return out_full



# revision 3
# speedup vs baseline: 1.2973x; 1.2973x over previous
"""GQA + sliding-window attention (B=2, S=2048, E=2048, HQ=16, HKV=4, D=128, W=512).

Sharding: 8 cores = 2 batches x 4 KV-head groups (tensor parallel).
Each core computes its batch's full sequence for one KV head + its 4 Q heads,
plus the (row-sharded) output projection partial; the host sums the 4 partials
per batch (the "all-reduce" done host-side) and adds bo.

v2 layout/schedule (vs v1):
  - k-outer projections accumulate K^T/Q^T/V^T into [128, S] PSUM tiles
    (4 banks each, 2 live), so matmuls start as soon as each xT k-tile's
    DMA lands instead of after the whole 14 MB input load.
  - V computed as V^T then turned into natural [s,d] via 16 PE transposes.
  - scores per k-tile into a double-buffered [128, 640] PSUM tile
    (512-col + 128-col matmuls, bank aligned), masked (DVE) + exp'd
    (ScalarE) without blocking the next k-tile's score matmul.
  - PV runs qi-major one tile behind exp, into a [128, 256] PSUM bank:
    cols 0:128 accumulate V^T E contributions, cols 128:256 accumulate
    the row-sum via an all-ones [k,128] stationary, which lands the
    softmax denominator replicated across all 128 partitions -- so
    normalization is just DVE reciprocal + multiply, no broadcast DMA.
  - output projection interleaved into the last head's loop; out is bf16.
"""

import os

import numpy as np
import ml_dtypes

import concourse.bass as bass
import concourse.mybir as mybir
import concourse.tile as tile
from concourse.tile import add_dep_helper
from concourse.bass_utils import run_bass_kernel_spmd

B, S, E = 2, 2048, 2048
HQ, HKV, D = 16, 4, 128
WINDOW = 512
ROPE_BASE = 10000.0
N_CORES = 8
GROUP = HQ // HKV          # 4 Q heads per KV head
HD_Q = GROUP * D           # 512
ST = S // 128              # 16 sequence tiles
KTILES = E // 128          # 16 contraction tiles over E
WT = WINDOW // 128         # 4 -> window spans WT+1 = 5 q-tiles
NEG = -30000.0

f32 = mybir.dt.float32
bf16 = mybir.dt.bfloat16


def _split_sync_waits(nc, max_waits=1):
    """walrus in this container rejects instructions with more than one
    sync-wait; split extras onto preceding same-engine NoOps."""
    for fn in nc.m.functions:
        for blk in fn.blocks:
            new_insts = []
            for inst in blk.instructions:
                si = getattr(inst, "sync_info", None)
                if si is not None and len(si.on_wait) > max_waits:
                    waits = list(si.on_wait)
                    head, tail = waits[:-max_waits], waits[-max_waits:]
                    for i in range(0, len(head), max_waits):
                        nop = mybir.InstNoOp(
                            name=f"splitwait-{nc.next_id()}",
                            ins=[], outs=[],
                            sync_info=mybir.SyncInfo(
                                on_wait=head[i:i + max_waits], on_update=[]),
                            bass_nofuse=True,
                        )
                        nop.engine = inst.engine
                        new_insts.append(nop)
                    inst.sync_info = mybir.SyncInfo(
                        on_wait=tail, on_update=list(si.on_update))
                new_insts.append(inst)
            blk.instructions[:] = new_insts


def build_kernel(has_bias):
    nc = bass.Bass("TRN2", target_bir_lowering=False, debug=False,
                   num_devices=N_CORES)
    Exp = mybir.ActivationFunctionType.Exp

    xT = nc.dram_tensor("xT", [E, S], bf16, kind="ExternalInput").ap()
    wqT = nc.dram_tensor("wqT", [E, HD_Q], bf16, kind="ExternalInput").ap()
    wkT = nc.dram_tensor("wkT", [E, D], bf16, kind="ExternalInput").ap()
    wvT = nc.dram_tensor("wvT", [E, D], bf16, kind="ExternalInput").ap()
    woT = nc.dram_tensor("woT", [HD_Q, E], bf16, kind="ExternalInput").ap()
    cosT = nc.dram_tensor("cosT", [D, S], f32, kind="ExternalInput").ap()
    sinT = nc.dram_tensor("sinT", [D, S], f32, kind="ExternalInput").ap()
    if has_bias:
        bqr = nc.dram_tensor("bqr", [1, HD_Q], bf16, kind="ExternalInput").ap()
        bkr = nc.dram_tensor("bkr", [1, D], bf16, kind="ExternalInput").ap()
        bvr = nc.dram_tensor("bvr", [1, D], bf16, kind="ExternalInput").ap()
    out = nc.dram_tensor("out", [S, E], bf16, kind="ExternalOutput").ap()

    with tile.TileContext(nc) as tc:
        with tc.tile_pool(name="singles", bufs=1) as singles, \
             tc.tile_pool(name="upool", bufs=4) as upool, \
             tc.tile_pool(name="epool", bufs=7) as epool, \
             tc.tile_pool(name="rbpool", bufs=3) as rbpool, \
             tc.tile_pool(name="ostage", bufs=4) as ostage:

            # ---- resident tensors ----
            xt = singles.tile([128, KTILES, S], bf16)
            wq = singles.tile([128, KTILES, HD_Q], bf16)
            wk = singles.tile([128, KTILES, D], bf16)
            wv = singles.tile([128, KTILES, D], bf16)
            wo = singles.tile([128, GROUP, E], bf16)
            cost = singles.tile([128, S], f32)
            sint = singles.tile([128, S], f32)
            qt = singles.tile([128, GROUP, S], bf16)
            kt = singles.tile([128, S], bf16)
            vtsb = singles.tile([128, S], bf16)
            vv = singles.tile([128, ST, D], bf16)
            ot = singles.tile([128, GROUP * ST, D], bf16)
            m0 = singles.tile([128, 128], f32)
            m4 = singles.tile([128, 128], f32)
            ones128 = singles.tile([128, 128], bf16)
            ident = singles.tile([128, 128], bf16)

            # interleave input DMAs so k-outer projections start early
            for t in range(KTILES):
                nc.sync.dma_start(out=xt[:, t, :], in_=xT[t * 128:(t + 1) * 128, :])
                nc.sync.dma_start(out=wk[:, t, :], in_=wkT[t * 128:(t + 1) * 128, :])
                nc.sync.dma_start(out=wq[:, t, :], in_=wqT[t * 128:(t + 1) * 128, :])
                nc.sync.dma_start(out=wv[:, t, :], in_=wvT[t * 128:(t + 1) * 128, :])
                if t == 8:
                    nc.sync.dma_start(out=cost[:], in_=cosT)
                    nc.sync.dma_start(out=sint[:], in_=sinT)
            for h in range(GROUP):
                nc.sync.dma_start(out=wo[:, h, :], in_=woT[h * 128:(h + 1) * 128, :])
            bq_t = bk_t = bv_t = onesrow = None
            if has_bias:
                bq_t = singles.tile([1, HD_Q], bf16)
                bk_t = singles.tile([1, D], bf16)
                bv_t = singles.tile([1, D], bf16)
                onesrow = singles.tile([1, 512], bf16)
                nc.sync.dma_start(out=bq_t[:], in_=bqr)
                nc.sync.dma_start(out=bk_t[:], in_=bkr)
                nc.sync.dma_start(out=bv_t[:], in_=bvr)
                nc.gpsimd.memset(onesrow[:], 1.0)

            # masks in S^T [k(p), q(x)] orientation:
            # diag tile: allowed iff q >= k  ->  x - p >= 0
            nc.gpsimd.memset(m0[:], 0.0)
            nc.gpsimd.affine_select(
                out=m0[:], in_=m0[:], compare_op=mybir.AluOpType.is_ge,
                fill=NEG, base=0, channel_multiplier=-1, pattern=[[1, 128]])
            # off-4 tile: allowed iff q <= k  ->  p - x >= 0
            nc.gpsimd.memset(m4[:], 0.0)
            nc.gpsimd.affine_select(
                out=m4[:], in_=m4[:], compare_op=mybir.AluOpType.is_ge,
                fill=NEG, base=0, channel_multiplier=1, pattern=[[-1, 128]])
            nc.gpsimd.memset(ones128[:], 1.0)
            nc.gpsimd.memset(ident[:], 1.0)
            nc.gpsimd.affine_select(
                out=ident[:], in_=ident[:], compare_op=mybir.AluOpType.is_equal,
                fill=0.0, base=0, channel_multiplier=-1, pattern=[[1, 128]])

            # ---- k-outer projections: accumulate [128, S] PSUM tiles ----
            def rope_drain(ps, dst):
                """dst[:, :] = rope(ps) where ps is a [128, S] psum view."""
                for n in range(S // 512):
                    sl = slice(n * 512, (n + 1) * 512)
                    u_t = upool.tile([128, 512], bf16, tag="u")
                    u_sh = upool.tile([128, 512], bf16, tag="ush")
                    nc.vector.tensor_mul(u_t[:], ps[:, sl], sint[:, sl])
                    nc.sync.dma_start(out=u_sh[0:64, :], in_=u_t[64:128, :])
                    nc.sync.dma_start(out=u_sh[64:128, :], in_=u_t[0:64, :])
                    nc.vector.tensor_mul(dst[:, sl], ps[:, sl], cost[:, sl])
                    nc.vector.tensor_add(dst[:, sl], dst[:, sl], u_sh[:])

            def copy_drain(ps, dst):
                for n in range(S // 512):
                    sl = slice(n * 512, (n + 1) * 512)
                    nc.scalar.copy(dst[:, sl], ps[:, sl])

            with tc.tile_pool(name="projp", bufs=2, space="PSUM") as projp:
                # (stationary slice, bias tile, drain fn, drain dst)
                def bqs(lo, hi):
                    return bq_t[:, lo:hi] if has_bias else None

                jobs = [
                    (lambda k: wk[:, k, :], bk_t, rope_drain, kt[:]),
                    (lambda k: wq[:, k, 0:128], bqs(0, 128),
                     rope_drain, qt[:, 0, :]),
                    (lambda k: wv[:, k, :], bv_t, copy_drain, vtsb[:]),
                    (lambda k: wq[:, k, 128:256], bqs(128, 256),
                     rope_drain, qt[:, 1, :]),
                    (lambda k: wq[:, k, 256:384], bqs(256, 384),
                     rope_drain, qt[:, 2, :]),
                    (lambda k: wq[:, k, 384:512], bqs(384, 512),
                     rope_drain, qt[:, 3, :]),
                ]
                for j0 in range(0, len(jobs), 2):
                    pair = jobs[j0:j0 + 2]
                    tiles = [projp.tile([128, S], f32, tag="pj",
                                        name=f"pj_{j0 + i}")
                             for i in range(len(pair))]
                    for k in range(KTILES):
                        for (stf, btile, _, _), pt in zip(pair, tiles):
                            for n in range(S // 512):
                                nc.tensor.matmul(
                                    pt[:, n * 512:(n + 1) * 512], stf(k),
                                    xt[:, k, n * 512:(n + 1) * 512],
                                    start=(k == 0),
                                    stop=(k == KTILES - 1 and btile is None))
                    if has_bias:
                        for (stf, btile, _, _), pt in zip(pair, tiles):
                            if btile is None:
                                continue
                            for n in range(S // 512):
                                nc.tensor.matmul(
                                    pt[:, n * 512:(n + 1) * 512], btile,
                                    onesrow[0:1, :], start=False, stop=True)
                    for (_, _, drain, dst), pt in zip(pair, tiles):
                        drain(pt, dst)

            # V^T -> V natural via PE transposes
            with tc.tile_pool(name="tpp", bufs=2, space="PSUM") as tpp:
                for sm in range(ST):
                    tp = tpp.tile([128, 128], bf16, tag="tp")
                    nc.tensor.transpose(
                        tp[:], vtsb[:, sm * 128:(sm + 1) * 128], ident[:])
                    nc.scalar.copy(vv[:, sm, :], tp[:])

            # ---- attention + (for last head) output projection ----
            with tc.tile_pool(name="score_psum", bufs=2, space="PSUM") as score_psum, \
                 tc.tile_pool(name="pv_psum", bufs=2, space="PSUM") as pv_psum, \
                 tc.tile_pool(name="out_psum", bufs=2, space="PSUM") as out_psum:

                def oproj(qi):
                    for ch in range(E // 512):
                        pso = out_psum.tile([128, 512], f32, tag="po2")
                        for h in range(GROUP):
                            nc.tensor.matmul(
                                pso[:], ot[:, h * ST + qi, :],
                                wo[:, h, ch * 512:(ch + 1) * 512],
                                start=(h == 0), stop=(h == GROUP - 1))
                        st = ostage.tile([128, 512], bf16, tag="st")
                        if ch % 2 == 0:
                            nc.vector.tensor_copy(st[:], pso[:])
                        else:
                            nc.scalar.copy(st[:], pso[:])
                        nc.sync.dma_start(
                            out=out[qi * 128:(qi + 1) * 128,
                                    ch * 512:(ch + 1) * 512],
                            in_=st[:])

                for m in range(GROUP):
                    e_tiles = {}

                    def pv_finish(qi):
                        """PV + rowsum for q-tile qi, then normalize into ot."""
                        kjs = list(range(max(0, qi - WT), qi + 1))
                        po = pv_psum.tile([128, 256], f32, tag="po",
                                          name=f"po_{m}_{qi}")
                        pv0 = None
                        for j, kjj in enumerate(kjs):
                            off = (qi - kjj) * 128
                            mm = nc.tensor.matmul(
                                po[:, 0:128], vv[:, kjj, :],
                                e_tiles[kjj][:, off:off + 128],
                                start=(j == 0), stop=(j == len(kjs) - 1))
                            if j == 0:
                                pv0 = mm
                        for j, kjj in enumerate(kjs):
                            off = (qi - kjj) * 128
                            mm = nc.tensor.matmul(
                                po[:, 128:256], ones128[:],
                                e_tiles[kjj][:, off:off + 128],
                                start=False, stop=(j == len(kjs) - 1),
                                skip_group_check=True)
                            if j == 0:
                                # rT group relies on pv0's start=True having
                                # cleared the bank's has_written bits first
                                add_dep_helper(mm.ins, pv0.ins, sync=False,
                                               reason="rT after bank clear")
                        rb = rbpool.tile([128, 128], f32, tag="rb")
                        nc.vector.reciprocal(rb[:], po[:, 128:256])
                        nc.vector.tensor_mul(
                            ot[:, m * ST + qi, :], po[:, 0:128], rb[:])

                    for kj in range(ST):
                        nw = min(WT + 1, ST - kj)
                        W = 128 * nw
                        q0 = kj * 128
                        pss = score_psum.tile([128, 640], f32, tag="ss")
                        n0 = min(W, 512)
                        nc.tensor.matmul(
                            pss[:, 0:n0], kt[:, q0:q0 + 128],
                            qt[:, m, q0:q0 + n0], start=True, stop=True)
                        if W > 512:
                            nc.tensor.matmul(
                                pss[:, 512:W], kt[:, q0:q0 + 128],
                                qt[:, m, q0 + 512:q0 + W], start=True, stop=True)
                        nc.vector.tensor_add(pss[:, 0:128], pss[:, 0:128], m0[:])
                        if nw == WT + 1:
                            nc.vector.tensor_add(
                                pss[:, 512:640], pss[:, 512:640], m4[:])
                        e_t = epool.tile([128, 640], bf16, tag="e")
                        nc.scalar.activation(e_t[:, 0:W], pss[:, 0:W], Exp)
                        e_tiles[kj] = e_t
                        if kj >= 1:
                            pv_finish(kj - 1)
                            if m == GROUP - 1 and kj >= 2:
                                oproj(kj - 2)
                    pv_finish(ST - 1)
                    if m == GROUP - 1:
                        oproj(ST - 2)
                        oproj(ST - 1)

    _split_sync_waits(nc)
    return nc


def _rope_tables():
    half = D // 2
    inv_freq = 1.0 / (ROPE_BASE ** (np.arange(half, dtype=np.float64) / half))
    ang = np.arange(S, dtype=np.float64)[:, None] * inv_freq[None, :]  # [S, 64]
    cos = np.cos(ang).T.astype(np.float32)          # [64, S]
    sin = np.sin(ang).T.astype(np.float32)
    cosT = np.concatenate([cos, cos], 0)            # [128, S]
    sinT = np.concatenate([sin, -sin], 0)           # sign-folded rotate-half
    return np.ascontiguousarray(cosT), np.ascontiguousarray(sinT)


def kernel(x, Wq, bq, Wk, bk, Wv, bv, Wo, bo, **kw):
    x = np.asarray(x, np.float32)
    Wq = np.asarray(Wq, np.float32); bq = np.asarray(bq, np.float32)
    Wk = np.asarray(Wk, np.float32); bk = np.asarray(bk, np.float32)
    Wv = np.asarray(Wv, np.float32); bv = np.asarray(bv, np.float32)
    Wo = np.asarray(Wo, np.float32); bo = np.asarray(bo, np.float32)

    has_bias = bool(np.any(bq) or np.any(bk) or np.any(bv))
    nc = build_kernel(has_bias)

    bff = ml_dtypes.bfloat16
    cosT, sinT = _rope_tables()
    scale = 1.0 / np.sqrt(np.float32(D))

    in_maps = []
    for c in range(N_CORES):
        b, h = c // HKV, c % HKV
        qs = slice(h * HD_Q, (h + 1) * HD_Q)
        ks = slice(h * D, (h + 1) * D)
        m = {
            "xT": np.ascontiguousarray(x[b].T).astype(bff),
            "wqT": np.ascontiguousarray(Wq[qs].T).astype(bff),
            "wkT": np.ascontiguousarray((Wk[ks] * scale).T).astype(bff),
            "wvT": np.ascontiguousarray(Wv[ks].T).astype(bff),
            "woT": np.ascontiguousarray(Wo[:, qs].T).astype(bff),
            "cosT": cosT,
            "sinT": sinT,
        }
        if has_bias:
            m["bqr"] = np.ascontiguousarray(bq[qs][None, :]).astype(bff)
            m["bkr"] = np.ascontiguousarray((bk[ks] * scale)[None, :]).astype(bff)
            m["bvr"] = np.ascontiguousarray(bv[ks][None, :]).astype(bff)
        in_maps.append(m)

    res = run_bass_kernel_spmd(nc, in_maps, core_ids=list(range(N_CORES)))
    global LAST_RESULT
    LAST_RESULT = res
    if os.environ.get("BASS_KERNEL_RETIME"):
        # executable is now cached in-process: a second run times
        # transfer + device execution without compile.
        import time
        t0 = time.time()
        run_bass_kernel_spmd(nc, in_maps, core_ids=list(range(N_CORES)))
        print(f"retime run (transfer+exec): {time.time()-t0:.3f}s")

    out_full = np.zeros((B, S, E), np.float32)
    for c in range(N_CORES):
        out_full[c // HKV] += res.results[c]["out"].astype(np.float32)
    out_full += bo[None, None, :]
    return out_full


# revision 10
# speedup vs baseline: 1.4339x; 1.1052x over previous
"""GQA + sliding-window attention (B=2, S=2048, E=2048, HQ=16, HKV=4, D=128, W=512).

Sharding: 8 cores = 2 batches x 4 KV-head groups (tensor parallel).
Each core computes its batch's full sequence for one KV head + its 4 Q heads,
plus the (row-sharded) output projection partial; the host sums the 4 partials
per batch (the "all-reduce" done host-side) and adds bo.

v2 layout/schedule (vs v1):
  - k-outer projections accumulate K^T/Q^T/V^T into [128, S] PSUM tiles
    (4 banks each, 2 live), so matmuls start as soon as each xT k-tile's
    DMA lands instead of after the whole 14 MB input load.
  - V computed as V^T then turned into natural [s,d] via 16 PE transposes.
  - scores per k-tile into a double-buffered [128, 640] PSUM tile
    (512-col + 128-col matmuls, bank aligned), masked (DVE) + exp'd
    (ScalarE) without blocking the next k-tile's score matmul.
  - PV runs qi-major one tile behind exp, into a [128, 256] PSUM bank:
    cols 0:128 accumulate V^T E contributions, cols 128:256 accumulate
    the row-sum via an all-ones [k,128] stationary, which lands the
    softmax denominator replicated across all 128 partitions -- so
    normalization is just DVE reciprocal + multiply, no broadcast DMA.
  - output projection interleaved into the last head's loop; out is bf16.
"""

import os

import numpy as np
import ml_dtypes

import concourse.bass as bass
import concourse.mybir as mybir
import concourse.tile as tile
from concourse.tile import add_dep_helper
from concourse.bass_utils import run_bass_kernel_spmd

B, S, E = 2, 2048, 2048
HQ, HKV, D = 16, 4, 128
WINDOW = 512
ROPE_BASE = 10000.0
N_CORES = 8
GROUP = HQ // HKV          # 4 Q heads per KV head
HD_Q = GROUP * D           # 512
ST = S // 128              # 16 sequence tiles
KTILES = E // 128          # 16 contraction tiles over E
WT = WINDOW // 128         # 4 -> window spans WT+1 = 5 q-tiles
NEG = -30000.0

f32 = mybir.dt.float32
bf16 = mybir.dt.bfloat16


def _split_sync_waits(nc, max_waits=1):
    """walrus in this container rejects instructions with more than one
    sync-wait; split extras onto preceding same-engine NoOps."""
    for fn in nc.m.functions:
        for blk in fn.blocks:
            new_insts = []
            for inst in blk.instructions:
                si = getattr(inst, "sync_info", None)
                if si is not None and len(si.on_wait) > max_waits:
                    waits = list(si.on_wait)
                    head, tail = waits[:-max_waits], waits[-max_waits:]
                    for i in range(0, len(head), max_waits):
                        nop = mybir.InstNoOp(
                            name=f"splitwait-{nc.next_id()}",
                            ins=[], outs=[],
                            sync_info=mybir.SyncInfo(
                                on_wait=head[i:i + max_waits], on_update=[]),
                            bass_nofuse=True,
                        )
                        nop.engine = inst.engine
                        new_insts.append(nop)
                    inst.sync_info = mybir.SyncInfo(
                        on_wait=tail, on_update=list(si.on_update))
                new_insts.append(inst)
            blk.instructions[:] = new_insts


def build_kernel(has_bias):
    nc = bass.Bass("TRN2", target_bir_lowering=False, debug=False,
                   num_devices=N_CORES)
    Exp = mybir.ActivationFunctionType.Exp

    xT = nc.dram_tensor("xT", [E, S], bf16, kind="ExternalInput").ap()
    wqT = nc.dram_tensor("wqT", [E, HD_Q], bf16, kind="ExternalInput").ap()
    wkT = nc.dram_tensor("wkT", [E, D], bf16, kind="ExternalInput").ap()
    wvT = nc.dram_tensor("wvT", [E, D], bf16, kind="ExternalInput").ap()
    woT = nc.dram_tensor("woT", [HD_Q, E], bf16, kind="ExternalInput").ap()
    cosT = nc.dram_tensor("cosT", [D, S], f32, kind="ExternalInput").ap()
    sinT = nc.dram_tensor("sinT", [D, S], f32, kind="ExternalInput").ap()
    if has_bias:
        bqr = nc.dram_tensor("bqr", [1, HD_Q], bf16, kind="ExternalInput").ap()
        bkr = nc.dram_tensor("bkr", [1, D], bf16, kind="ExternalInput").ap()
        bvr = nc.dram_tensor("bvr", [1, D], bf16, kind="ExternalInput").ap()
    out = nc.dram_tensor("out", [S, E], bf16, kind="ExternalOutput").ap()

    with tile.TileContext(nc) as tc:
        with tc.tile_pool(name="singles", bufs=1) as singles, \
             tc.tile_pool(name="upool", bufs=4) as upool, \
             tc.tile_pool(name="epool", bufs=7) as epool, \
             tc.tile_pool(name="rbpool", bufs=3) as rbpool, \
             tc.tile_pool(name="ostage", bufs=4) as ostage:

            # ---- resident tensors ----
            xt = singles.tile([128, KTILES, S], bf16)
            wq = singles.tile([128, KTILES, HD_Q], bf16)
            wk = singles.tile([128, KTILES, D], bf16)
            wv = singles.tile([128, KTILES, D], bf16)
            wo = singles.tile([128, GROUP, E], bf16)
            cost = singles.tile([128, S], f32)
            sint = singles.tile([128, S], f32)
            qt = singles.tile([128, GROUP, S], bf16)
            kt = singles.tile([128, S], bf16)
            vtsb = singles.tile([128, S], bf16)
            vv = singles.tile([128, ST, D], bf16)
            ot = singles.tile([128, GROUP * ST, D], bf16)
            mt_diag = singles.tile([128, 128], bf16)
            mt_off4 = singles.tile([128, 128], bf16)
            ones128 = singles.tile([128, 128], bf16)
            ident = singles.tile([128, 128], bf16)

            # interleave input DMAs so k-outer projections start early
            for t in range(KTILES):
                nc.sync.dma_start(out=xt[:, t, :], in_=xT[t * 128:(t + 1) * 128, :])
                nc.sync.dma_start(out=wk[:, t, :], in_=wkT[t * 128:(t + 1) * 128, :])
                nc.sync.dma_start(out=wq[:, t, :], in_=wqT[t * 128:(t + 1) * 128, :])
                nc.sync.dma_start(out=wv[:, t, :], in_=wvT[t * 128:(t + 1) * 128, :])
                if t == 8:
                    nc.sync.dma_start(out=cost[:], in_=cosT)
                    nc.sync.dma_start(out=sint[:], in_=sinT)
            for h in range(GROUP):
                nc.sync.dma_start(out=wo[:, h, :], in_=woT[h * 128:(h + 1) * 128, :])
            bq_t = bk_t = bv_t = onesrow = None
            if has_bias:
                bq_t = singles.tile([1, HD_Q], bf16)
                bk_t = singles.tile([1, D], bf16)
                bv_t = singles.tile([1, D], bf16)
                onesrow = singles.tile([1, 512], bf16)
                nc.sync.dma_start(out=bq_t[:], in_=bqr)
                nc.sync.dma_start(out=bk_t[:], in_=bkr)
                nc.sync.dma_start(out=bv_t[:], in_=bvr)
                nc.gpsimd.memset(onesrow[:], 1.0)

            # masks are ADDED to the scores psum via an extra matmul with
            # moving=identity: out[m,n] += st[n,m], so each stationary holds
            # the TRANSPOSE of the mask to apply in S^T [k(p), q(x)] coords.
            # diag tile mask M[k,q] = NEG where q < k -> st[q,k] = NEG where
            # p < x: keep where p - x >= 0.
            nc.gpsimd.memset(mt_diag[:], 0.0)
            nc.gpsimd.affine_select(
                out=mt_diag[:], in_=mt_diag[:], compare_op=mybir.AluOpType.is_ge,
                fill=NEG, base=0, channel_multiplier=1, pattern=[[-1, 128]])
            # off-4 tile mask M[k,q] = NEG where q > k -> st[q,k] = NEG where
            # p > x: keep where x - p >= 0.
            nc.gpsimd.memset(mt_off4[:], 0.0)
            nc.gpsimd.affine_select(
                out=mt_off4[:], in_=mt_off4[:], compare_op=mybir.AluOpType.is_ge,
                fill=NEG, base=0, channel_multiplier=-1, pattern=[[1, 128]])
            nc.gpsimd.memset(ones128[:], 1.0)
            nc.gpsimd.memset(ident[:], 1.0)
            nc.gpsimd.affine_select(
                out=ident[:], in_=ident[:], compare_op=mybir.AluOpType.is_equal,
                fill=0.0, base=0, channel_multiplier=-1, pattern=[[1, 128]])

            # ---- projections ----
            def rope_chunk(ps, dst, sl):
                """dst[:, sl] = rope(ps[:, psl]); ps is a psum view whose
                columns already correspond to dst's slice sl."""
                u_t = upool.tile([128, 512], bf16, tag="u")
                u_sh = upool.tile([128, 512], bf16, tag="ush")
                nc.vector.tensor_mul(u_t[:], ps, sint[:, sl])
                nc.sync.dma_start(out=u_sh[0:64, :], in_=u_t[64:128, :])
                nc.sync.dma_start(out=u_sh[64:128, :], in_=u_t[0:64, :])
                nc.vector.tensor_mul(dst[:, sl], ps, cost[:, sl])
                nc.vector.tensor_add(dst[:, sl], dst[:, sl], u_sh[:])

            def bqs(lo, hi):
                return bq_t[:, lo:hi] if has_bias else None

            # P1: K and Q0 k-outer over full-width [128, S] psum tiles, so
            # matmuls chase the xT tile DMAs as they land.
            with tc.tile_pool(name="projp", bufs=2, space="PSUM") as projp:
                p1 = [
                    (lambda k: wk[:, k, :], bk_t, kt[:]),
                    (lambda k: wq[:, k, 0:128], bqs(0, 128), qt[:, 0, :]),
                ]
                tiles = [projp.tile([128, S], f32, tag="pj", name=f"pj_{i}")
                         for i in range(2)]
                for k in range(KTILES):
                    for (stf, btile, _), pt in zip(p1, tiles):
                        for n in range(S // 512):
                            nc.tensor.matmul(
                                pt[:, n * 512:(n + 1) * 512], stf(k),
                                xt[:, k, n * 512:(n + 1) * 512],
                                start=(k == 0),
                                stop=(k == KTILES - 1 and btile is None))
                if has_bias:
                    for (stf, btile, _), pt in zip(p1, tiles):
                        for n in range(S // 512):
                            nc.tensor.matmul(
                                pt[:, n * 512:(n + 1) * 512], btile,
                                onesrow[0:1, :], start=False, stop=True)
                for (_, _, dst), pt in zip(p1, tiles):
                    for n in range(S // 512):
                        sl = slice(n * 512, (n + 1) * 512)
                        rope_chunk(pt[:, sl], dst, sl)

            # P2: V^T, Q1-Q3 as 512-col chunks through a 4-deep psum pool;
            # chunk drains (and V transposes, lagged one chunk) overlap the
            # next chunk's accumulation.
            with tc.tile_pool(name="proj2", bufs=4, space="PSUM") as proj2:

                def transpose_chunk(c):
                    for t in range(4):
                        sm = c * 4 + t
                        tp = proj2.tile([128, 128], bf16, tag="tp")
                        nc.tensor.transpose(
                            tp[:], vtsb[:, sm * 128:(sm + 1) * 128], ident[:])
                        nc.scalar.copy(vv[:, sm, :], tp[:])

                p2 = [
                    (lambda k: wv[:, k, :], bv_t, None),
                    (lambda k: wq[:, k, 128:256], bqs(128, 256), qt[:, 1, :]),
                    (lambda k: wq[:, k, 256:384], bqs(256, 384), qt[:, 2, :]),
                    (lambda k: wq[:, k, 384:512], bqs(384, 512), qt[:, 3, :]),
                ]
                for ji, (stf, btile, dst) in enumerate(p2):
                    for c in range(S // 512):
                        sl = slice(c * 512, (c + 1) * 512)
                        pc = proj2.tile([128, 512], f32, tag="pc")
                        for k in range(KTILES):
                            nc.tensor.matmul(
                                pc[:], stf(k), xt[:, k, sl],
                                start=(k == 0),
                                stop=(k == KTILES - 1 and btile is None))
                        if has_bias:
                            nc.tensor.matmul(
                                pc[:], btile, onesrow[0:1, :],
                                start=False, stop=True)
                        if ji == 0:
                            nc.scalar.copy(vtsb[:, sl], pc[:])
                            if c >= 1:
                                transpose_chunk(c - 1)
                        else:
                            rope_chunk(pc[:], dst, sl)
                    if ji == 0:
                        transpose_chunk(3)

            # ---- attention + (for last head) output projection ----
            with tc.tile_pool(name="score_psum", bufs=2, space="PSUM") as score_psum, \
                 tc.tile_pool(name="pv_psum", bufs=2, space="PSUM") as pv_psum, \
                 tc.tile_pool(name="out_psum", bufs=2, space="PSUM") as out_psum:

                def oproj(qi):
                    for ch in range(E // 512):
                        pso = out_psum.tile([128, 512], f32, tag="po2")
                        for h in range(GROUP):
                            nc.tensor.matmul(
                                pso[:], ot[:, h * ST + qi, :],
                                wo[:, h, ch * 512:(ch + 1) * 512],
                                start=(h == 0), stop=(h == GROUP - 1))
                        st = ostage.tile([128, 512], bf16, tag="st")
                        if ch % 2 == 0:
                            nc.vector.tensor_copy(st[:], pso[:])
                        else:
                            nc.scalar.copy(st[:], pso[:])
                        nc.sync.dma_start(
                            out=out[qi * 128:(qi + 1) * 128,
                                    ch * 512:(ch + 1) * 512],
                            in_=st[:])

                for m in range(GROUP):
                    e_tiles = {}

                    def pv_finish(qi):
                        """PV + rowsum for q-tile qi, then normalize into ot."""
                        kjs = list(range(max(0, qi - WT), qi + 1))
                        po = pv_psum.tile([128, 256], f32, tag="po",
                                          name=f"po_{m}_{qi}")
                        pv0 = None
                        for j, kjj in enumerate(kjs):
                            off = (qi - kjj) * 128
                            mm = nc.tensor.matmul(
                                po[:, 0:128], vv[:, kjj, :],
                                e_tiles[kjj][:, off:off + 128],
                                start=(j == 0), stop=(j == len(kjs) - 1))
                            if j == 0:
                                pv0 = mm
                        for j, kjj in enumerate(kjs):
                            off = (qi - kjj) * 128
                            mm = nc.tensor.matmul(
                                po[:, 128:256], ones128[:],
                                e_tiles[kjj][:, off:off + 128],
                                start=False, stop=(j == len(kjs) - 1),
                                skip_group_check=True)
                            if j == 0:
                                # rT group relies on pv0's start=True having
                                # cleared the bank's has_written bits first
                                add_dep_helper(mm.ins, pv0.ins, sync=False,
                                               reason="rT after bank clear")
                        rb = rbpool.tile([128, 128], f32, tag="rb")
                        nc.vector.reciprocal(rb[:], po[:, 128:256])
                        nc.vector.tensor_mul(
                            ot[:, m * ST + qi, :], po[:, 0:128], rb[:])

                    for kj in range(ST):
                        nw = min(WT + 1, ST - kj)
                        W = 128 * nw
                        q0 = kj * 128
                        pss = score_psum.tile([128, 640], f32, tag="ss")
                        n0 = min(W, 512)
                        sa = nc.tensor.matmul(
                            pss[:, 0:n0], kt[:, q0:q0 + 128],
                            qt[:, m, q0:q0 + n0], start=True, stop=False)
                        ma = nc.tensor.matmul(
                            pss[:, 0:128], mt_diag[:], ident[:],
                            start=False, stop=True, skip_group_check=True)
                        add_dep_helper(ma.ins, sa.ins, sync=False,
                                       reason="mask add after score write")
                        if W > 512:
                            sb = nc.tensor.matmul(
                                pss[:, 512:W], kt[:, q0:q0 + 128],
                                qt[:, m, q0 + 512:q0 + W], start=True, stop=False)
                            mb = nc.tensor.matmul(
                                pss[:, 512:640], mt_off4[:], ident[:],
                                start=False, stop=True, skip_group_check=True)
                            add_dep_helper(mb.ins, sb.ins, sync=False,
                                           reason="mask add after score write")
                        e_t = epool.tile([128, 640], bf16, tag="e")
                        nc.scalar.activation(e_t[:, 0:W], pss[:, 0:W], Exp)
                        e_tiles[kj] = e_t
                        if kj >= 1:
                            pv_finish(kj - 1)
                            if m == GROUP - 1 and kj >= 2:
                                oproj(kj - 2)
                    pv_finish(ST - 1)
                    if m == GROUP - 1:
                        oproj(ST - 2)
                        oproj(ST - 1)

    _split_sync_waits(nc)
    return nc


def _rope_tables():
    half = D // 2
    inv_freq = 1.0 / (ROPE_BASE ** (np.arange(half, dtype=np.float64) / half))
    ang = np.arange(S, dtype=np.float64)[:, None] * inv_freq[None, :]  # [S, 64]
    cos = np.cos(ang).T.astype(np.float32)          # [64, S]
    sin = np.sin(ang).T.astype(np.float32)
    cosT = np.concatenate([cos, cos], 0)            # [128, S]
    sinT = np.concatenate([sin, -sin], 0)           # sign-folded rotate-half
    return np.ascontiguousarray(cosT), np.ascontiguousarray(sinT)


def kernel(x, Wq, bq, Wk, bk, Wv, bv, Wo, bo, **kw):
    x = np.asarray(x, np.float32)
    Wq = np.asarray(Wq, np.float32); bq = np.asarray(bq, np.float32)
    Wk = np.asarray(Wk, np.float32); bk = np.asarray(bk, np.float32)
    Wv = np.asarray(Wv, np.float32); bv = np.asarray(bv, np.float32)
    Wo = np.asarray(Wo, np.float32); bo = np.asarray(bo, np.float32)

    has_bias = bool(np.any(bq) or np.any(bk) or np.any(bv))
    nc = build_kernel(has_bias)

    bff = ml_dtypes.bfloat16
    cosT, sinT = _rope_tables()
    scale = 1.0 / np.sqrt(np.float32(D))

    in_maps = []
    for c in range(N_CORES):
        b, h = c // HKV, c % HKV
        qs = slice(h * HD_Q, (h + 1) * HD_Q)
        ks = slice(h * D, (h + 1) * D)
        m = {
            "xT": np.ascontiguousarray(x[b].T).astype(bff),
            "wqT": np.ascontiguousarray(Wq[qs].T).astype(bff),
            "wkT": np.ascontiguousarray((Wk[ks] * scale).T).astype(bff),
            "wvT": np.ascontiguousarray(Wv[ks].T).astype(bff),
            "woT": np.ascontiguousarray(Wo[:, qs].T).astype(bff),
            "cosT": cosT,
            "sinT": sinT,
        }
        if has_bias:
            m["bqr"] = np.ascontiguousarray(bq[qs][None, :]).astype(bff)
            m["bkr"] = np.ascontiguousarray((bk[ks] * scale)[None, :]).astype(bff)
            m["bvr"] = np.ascontiguousarray(bv[ks][None, :]).astype(bff)
        in_maps.append(m)

    res = run_bass_kernel_spmd(nc, in_maps, core_ids=list(range(N_CORES)))
    global LAST_RESULT
    LAST_RESULT = res
    if os.environ.get("BASS_KERNEL_RETIME"):
        # executable is now cached in-process: a second run times
        # transfer + device execution without compile.
        import time
        t0 = time.time()
        run_bass_kernel_spmd(nc, in_maps, core_ids=list(range(N_CORES)))
        print(f"retime run (transfer+exec): {time.time()-t0:.3f}s")

    out_full = np.zeros((B, S, E), np.float32)
    for c in range(N_CORES):
        out_full[c // HKV] += res.results[c]["out"].astype(np.float32)
    out_full += bo[None, None, :]
    return out_full
